# revision 6
# baseline (speedup 1.0000x reference)
"""LoRA linear kernel for Trainium2 (Bass/Tile), 8-core SPMD.

Computes out = x @ (A @ B) * (alpha/r) for
  x: [4, 4096, 4096] f32, A: [4096, 16] f32, B: [16, 4096] f32
with alpha/r == 1.0.

Algorithm: reassociate as out = (x @ A) @ B  -- 128x fewer FLOPs than
materializing the 4096x4096 delta-weight.  Data-parallel over rows of x:
each of the 8 cores gets 2048 rows.

The kernel is HBM-bandwidth bound (~358 GB/s per core), so all large
tensors move as bf16 (rel err ~4e-3, well under the 2e-2 gate):
  - x is cast to bf16 and pre-transposed/tiled per shard on the host, so
    the device streams xT straight into matmul1 (no on-device transpose)
    with every input DMA a single fully-contiguous 1 MiB block.
  - the output is produced as bf16 on device and upcast on the host.
HBM traffic per core: 16 MiB in + 16 MiB out (vs 64 MiB all-f32).

To keep the HBM stream saturated end-to-end, the m-rows are processed in
two halves with software pipelining: while half h streams out (phase 2:
t @ B, PSUM->SBUF bf16 copies, out DMAs), half h+1 streams in (phase 1:
xT chunks, tT[16, m] accumulation in PSUM).  The in/out DMA streams
interleave on the sync HWDGE ring so HBM never idles between phases.

Input tiling: xt_pre[h*8+s] = [128, 4096] where column cc*1024+j of
partition p holds xT[(4s+cc)*128 + p, h*1024 + j]; one segment = 4
k-chunks of one m-half = 1 MiB contiguous.
"""

import os
import sys

import numpy as np

for _p in ("/opt/trn_rl_repo",):
    if os.path.isdir(_p) and _p not in sys.path:
        sys.path.insert(0, _p)

import concourse.bacc as bacc
import concourse.mybir as mybir
from concourse import tile
from concourse.bass_utils import run_bass_kernel_spmd

import ml_dtypes

R = 16
B_DIM = 4
SEQ = 4096
K = 4096  # in_features
N = 4096  # out_features
M_FULL = B_DIM * SEQ  # 16384
NCORES = 8
M_SHARD = M_FULL // NCORES  # 2048
SCALING = 16.0 / 16.0  # alpha / r == 1.0

KC = 128  # contraction chunk (partition dim of xT tiles)
N_KC = K // KC  # 32
MH = M_SHARD // 2  # 1024, m-half
CC_SEG = 4  # k-chunks per input segment
N_SEG = N_KC // CC_SEG  # 8 segments per half
MM1_N = 512  # matmul1 moving free dim
MT = 128  # rows per m-tile in phase 2
N_MT_H = MH // MT  # 8 m-tiles per half
N_CHUNK = 512  # matmul2 moving free dim (one PSUM bank of fp32)
OPS_W = 1024  # psum out tile width (2 banks); copied in one ACT/DVE op

_F32 = mybir.dt.float32
_BF16 = mybir.dt.bfloat16


def _build_kernel(tc, nc, xt, a_pre, b_in, out):
    with (
        tc.tile_pool(name="const", bufs=1) as cpool,
        tc.tile_pool(name="xin", bufs=3) as xpool,
        tc.tile_pool(name="tps", bufs=2, space="PSUM") as tpsum,
        tc.tile_pool(name="tsb", bufs=2) as tspool,
        tc.tile_pool(name="ops", bufs=2, space="PSUM") as opsum,
        tc.tile_pool(name="osb", bufs=3) as opool,
    ):
        # A pre-arranged on host to [128, N_KC * R] bf16: col block c holds
        # A[c*128:(c+1)*128, :] with k on partitions.
        a_sb = cpool.tile([128, N_KC * R], _BF16, name="a_sb")
        nc.sync.dma_start(out=a_sb, in_=a_pre)
        b_sb = cpool.tile([R, N], _BF16, name="b_sb")
        nc.sync.dma_start(out=b_sb, in_=b_in)

        def phase1_segment(h, s, tps):
            """DMA one 1 MiB input segment; 8 accumulating matmuls."""
            xtile = xpool.tile([128, CC_SEG * MH], _BF16)
            nc.sync.dma_start(out=xtile, in_=xt[h * N_SEG + s : h * N_SEG + s + 1, :, :])
            for cc in range(CC_SEG):
                c = s * CC_SEG + cc
                for jj in range(MH // MM1_N):
                    nc.tensor.matmul(
                        tps[:, jj * MM1_N : (jj + 1) * MM1_N],
                        a_sb[:, c * R : (c + 1) * R],
                        xtile[:, cc * MH + jj * MM1_N : cc * MH + (jj + 1) * MM1_N],
                        start=(c == 0),
                        stop=(c == N_KC - 1),
                    )

        def phase2_mtile(h, mt, ts):
            """out[m-tile, :] = ts[:, m-tile].T @ B; copies + 1 MiB out DMA."""
            osb = opool.tile([MT, N], _BF16)
            for half_n in range(N // OPS_W):  # 4 psum tiles of [128, 1024]
                ops = opsum.tile([MT, OPS_W], _F32)
                for jj in range(OPS_W // N_CHUNK):
                    j = half_n * (OPS_W // N_CHUNK) + jj
                    nc.tensor.matmul(
                        ops[:, jj * N_CHUNK : (jj + 1) * N_CHUNK],
                        ts[:, mt * MT : (mt + 1) * MT],
                        b_sb[:, j * N_CHUNK : (j + 1) * N_CHUNK],
                        start=True,
                        stop=True,
                    )
                dst = osb[:, half_n * OPS_W : (half_n + 1) * OPS_W]
                if half_n % 2 == 0:
                    nc.scalar.copy(dst, ops[:])
                else:
                    nc.vector.tensor_copy(dst, ops[:])
            row0 = h * MH + mt * MT
            nc.sync.dma_start(out=out[row0 : row0 + MT, :], in_=osb[:])

        # Prologue: phase 1 of half 0.
        tps0 = tpsum.tile([R, MH], _F32, tag="tps")
        for s in range(N_SEG):
            phase1_segment(0, s, tps0)
        ts0 = tspool.tile([R, MH], _BF16, tag="ts")
        nc.vector.tensor_copy(ts0[:], tps0[:])

        # Steady: half 1 phase 1 interleaved with half 0 phase 2.  The
        # input stream leads by 2 segments so the first out-DMA's
        # semaphore wait (sync ring is in-order) never starves the
        # input DMAs.
        tps1 = tpsum.tile([R, MH], _F32, tag="tps")
        phase1_segment(1, 0, tps1)
        phase1_segment(1, 1, tps1)
        for s in range(N_SEG):
            phase2_mtile(0, s, ts0)
            if s + 2 < N_SEG:
                phase1_segment(1, s + 2, tps1)
        ts1 = tspool.tile([R, MH], _BF16, tag="ts")
        nc.vector.tensor_copy(ts1[:], tps1[:])

        # Epilogue: phase 2 of half 1.
        for mt in range(N_MT_H):
            phase2_mtile(1, mt, ts1)


_NC_CACHE = None


def _get_nc():
    global _NC_CACHE
    if _NC_CACHE is not None:
        return _NC_CACHE
    nc = bacc.Bacc("TRN2", target_bir_lowering=False, debug=False)
    xt = nc.dram_tensor(
        "xt", [2 * N_SEG, 128, CC_SEG * MH], _BF16, kind="ExternalInput"
    ).ap()
    a_pre = nc.dram_tensor("a_pre", [128, N_KC * R], _BF16, kind="ExternalInput").ap()
    b_in = nc.dram_tensor("b_in", [R, N], _BF16, kind="ExternalInput").ap()
    out = nc.dram_tensor("out", [M_SHARD, N], _BF16, kind="ExternalOutput").ap()
    with tile.TileContext(nc) as tc:
        _build_kernel(tc, nc, xt, a_pre, b_in, out)
    nc.compile()
    _NC_CACHE = nc
    return nc


LAST_RESULTS = None


def kernel(x: np.ndarray, A: np.ndarray, B: np.ndarray) -> np.ndarray:
    global LAST_RESULTS
    assert x.shape == (B_DIM, SEQ, K), x.shape
    assert A.shape == (K, R), A.shape
    assert B.shape == (R, N), B.shape

    x_bf = np.asarray(x, dtype=np.float32).reshape(M_FULL, K).astype(ml_dtypes.bfloat16)
    a_np = np.asarray(A, dtype=np.float32)
    b_np = np.ascontiguousarray(
        (np.asarray(B, dtype=np.float32) * SCALING).astype(ml_dtypes.bfloat16)
    )

    # Host pre-arrangement of A: [K, R] -> [128, (K/128) * R] bf16
    a_pre = np.ascontiguousarray(
        a_np.reshape(K // KC, KC, R).transpose(1, 0, 2).reshape(128, N_KC * R)
    ).astype(ml_dtypes.bfloat16)

    in_maps = []
    for i in range(NCORES):
        xT = x_bf[i * M_SHARD : (i + 1) * M_SHARD, :].T  # [K, M_SHARD]
        # [K, M] -> [s, cc, p, h, j] -> [h, s, p, cc, j] -> [16, 128, 4096]
        xt_i = np.ascontiguousarray(
            xT.reshape(N_SEG, CC_SEG, KC, 2, MH)
            .transpose(3, 0, 2, 1, 4)
            .reshape(2 * N_SEG, 128, CC_SEG * MH)
        )
        in_maps.append({"xt": xt_i, "a_pre": a_pre, "b_in": b_np})

    nc = _get_nc()
    trace = os.environ.get("KERNEL_TRACE", "0") == "1"
    tmpdir = os.environ.get("KERNEL_TMPDIR") or None
    res = run_bass_kernel_spmd(
        nc, in_maps, core_ids=list(range(NCORES)), trace=trace, tmpdir=tmpdir
    )
    LAST_RESULTS = res
    out = np.concatenate(
        [np.asarray(res.results[i]["out"], dtype=np.float32) for i in range(NCORES)],
        axis=0,
    )
    return out.reshape(B_DIM, SEQ, N)


# revision 8
# speedup vs baseline: 1.0872x; 1.0872x over previous
"""LoRA linear kernel for Trainium2 (Bass/Tile), 8-core SPMD.

Computes out = x @ (A @ B) * (alpha/r) for
  x: [4, 4096, 4096] f32, A: [4096, 16] f32, B: [16, 4096] f32
with alpha/r == 1.0.

Algorithm: reassociate as out = (x @ A) @ B  -- 128x fewer FLOPs than
materializing the 4096x4096 delta-weight.  Data-parallel over rows of x:
each of the 8 cores gets 2048 rows.

The kernel is HBM-bandwidth bound (~358 GB/s per core), so all large
tensors move as bf16 (rel err ~4e-3, well under the 2e-2 gate):
  - x is cast to bf16 and pre-transposed/tiled per shard on the host, so
    the device streams xT straight into matmul1 (no on-device transpose)
    with every input DMA a single fully-contiguous 1 MiB block.
  - the output is produced as bf16 on device and upcast on the host.
HBM traffic per core: 16 MiB in + 16 MiB out (vs 64 MiB all-f32).

The PE often sits at the cold 1.2 GHz HAM clock (DMA pacing leaves idle
windows, so it keeps re-throttling).  Untiled, the two matmul passes
cost ~109 us at 1.2 GHz -- above the ~90 us DMA floor.  Both matmuls
badly underuse the 128x128 array (mm1: lhs free dim 16; mm2: contract
dim 16), so each runs as TWO concurrent array tiles (tile_position):
  - mm1: 128x64 column tiling; lane 0 (cols 0-63) takes even k-chunks,
    lane 1 (cols 64-127) odd k-chunks.  Lane partials land at PSUM
    partitions 0-15 / 64-79 and one DVE add (+bf16 cast) combines them.
  - mm2: 64x128 row tiling; tT/B replicated at partitions 0-15 / 64-79,
    lane 0 takes n-chunks 0-3, lane 1 chunks 4-7.
This halves PE stream time (~56 us cold, ~28 us warm) so the PE never
paces the DMA streams.

To keep the HBM stream saturated end-to-end, the m-rows are processed in
two halves with software pipelining: while half h streams out (phase 2),
half h+1 streams in (phase 1); the in/out DMA streams interleave on the
sync HWDGE ring so HBM never idles between phases.

Input tiling: xt_pre[h*8+s] = [128, 4096] where column cc*1024+j of
partition p holds xT[(4s+cc)*128 + p, h*1024 + j]; one segment = 4
k-chunks of one m-half = 1 MiB contiguous.
"""

import os
import sys

import numpy as np

for _p in ("/opt/trn_rl_repo",):
    if os.path.isdir(_p) and _p not in sys.path:
        sys.path.insert(0, _p)

import concourse.bacc as bacc
import concourse.mybir as mybir
from concourse import tile
from concourse.alu_op_type import AluOpType
from concourse.bass_utils import run_bass_kernel_spmd

import ml_dtypes

R = 16
B_DIM = 4
SEQ = 4096
K = 4096  # in_features
N = 4096  # out_features
M_FULL = B_DIM * SEQ  # 16384
NCORES = 8
M_SHARD = M_FULL // NCORES  # 2048
SCALING = 16.0 / 16.0  # alpha / r == 1.0

KC = 128  # contraction chunk (partition dim of xT tiles)
N_KC = K // KC  # 32
MH = M_SHARD // 2  # 1024, m-half
CC_SEG = 4  # k-chunks per input segment
N_SEG = N_KC // CC_SEG  # 8 segments per half
MM1_N = 512  # matmul1 moving free dim
MT = 128  # rows per m-tile in phase 2
N_MT_H = MH // MT  # 8 m-tiles per half
N_CHUNK = 512  # matmul2 moving free dim (one PSUM bank of fp32)
N_NC = N // N_CHUNK  # 8
LANE_P = 64  # partition offset of array lane 1 (2x tiling)

_F32 = mybir.dt.float32
_BF16 = mybir.dt.bfloat16


def _build_kernel(tc, nc, xt, a_pre, b_in, out):
    with (
        tc.tile_pool(name="const", bufs=1) as cpool,
        tc.tile_pool(name="xin", bufs=3) as xpool,
        tc.tile_pool(name="tps", bufs=2, space="PSUM") as tpsum,
        tc.tile_pool(name="tsb", bufs=2) as tspool,
        tc.tile_pool(name="ops", bufs=4, space="PSUM") as opsum,
        tc.tile_pool(name="osb", bufs=3) as opool,
    ):
        # A pre-arranged on host to [128, N_KC * R] bf16: col block c holds
        # A[c*128:(c+1)*128, :] with k on partitions.
        a_sb = cpool.tile([128, N_KC * R], _BF16, name="a_sb")
        nc.sync.dma_start(out=a_sb, in_=a_pre)
        # B replicated on host at partition rows 0-15 and 64-79 (for the
        # two mm2 row-tiling lanes).
        b_sb = cpool.tile([LANE_P + R, N], _BF16, name="b_sb")
        nc.sync.dma_start(out=b_sb, in_=b_in)

        def phase1_segment(h, s, tps):
            """DMA one 1 MiB input segment; 8 accumulating matmuls on two
            column-tiled PE lanes (even k-chunks -> lane 0, odd -> 1)."""
            xtile = xpool.tile([128, CC_SEG * MH], _BF16)
            nc.sync.dma_start(out=xtile, in_=xt[h * N_SEG + s : h * N_SEG + s + 1, :, :])
            for cc in range(CC_SEG):
                c = s * CC_SEG + cc
                lane = c % 2
                p0 = lane * LANE_P
                for jj in range(MH // MM1_N):
                    nc.tensor.matmul(
                        tps[p0 : p0 + R, jj * MM1_N : (jj + 1) * MM1_N],
                        a_sb[:, c * R : (c + 1) * R],
                        xtile[:, cc * MH + jj * MM1_N : cc * MH + (jj + 1) * MM1_N],
                        start=(c == lane),
                        stop=(c == N_KC - 2 + lane),
                        tile_position=(0, p0),
                    )

        def phase2_mtile(h, mt, ts):
            """out[m-tile, :] = ts[:, m-tile].T @ B on two row-tiled PE
            lanes; PSUM->SBUF bf16 copies alternate ACT/DVE; 1 MiB out DMA."""
            osb = opool.tile([MT, N], _BF16)
            for jj in range(N_NC // 2):
                for lane in range(2):
                    j = lane * (N_NC // 2) + jj
                    p0 = lane * LANE_P
                    ops = opsum.tile([MT, N_CHUNK], _F32)
                    nc.tensor.matmul(
                        ops[:],
                        ts[p0 : p0 + R, mt * MT : (mt + 1) * MT],
                        b_sb[p0 : p0 + R, j * N_CHUNK : (j + 1) * N_CHUNK],
                        start=True,
                        stop=True,
                        tile_position=(p0, 0),
                    )
                    dst = osb[:, j * N_CHUNK : (j + 1) * N_CHUNK]
                    if jj % 2 == 0:
                        nc.scalar.copy(dst, ops[:])
                    else:
                        nc.vector.tensor_copy(dst, ops[:])
            row0 = h * MH + mt * MT
            nc.sync.dma_start(out=out[row0 : row0 + MT, :], in_=osb[:])

        def reduce_cast(tps):
            """Combine the two mm1 lane partials into bf16 tT, replicated
            at partitions 0-15 and 64-79 for the mm2 lanes.  (DVE can read
            only one input from PSUM, so lane 1's partial goes via SBUF.)"""
            tmp = tspool.tile([R, MH], _F32, tag="tred")
            nc.scalar.copy(tmp[:], tps[LANE_P : LANE_P + R, :])
            ts = tspool.tile([LANE_P + R, MH], _BF16, tag="ts")
            nc.vector.tensor_tensor(ts[0:R, :], tps[0:R, :], tmp[:], op=AluOpType.add)
            nc.scalar.copy(ts[LANE_P : LANE_P + R, :], ts[0:R, :])
            return ts

        # Prologue: phase 1 of half 0.
        tps0 = tpsum.tile([128, MH], _F32, tag="tps")
        for s in range(N_SEG):
            phase1_segment(0, s, tps0)
        ts0 = reduce_cast(tps0)

        # Steady: half 1 phase 1 interleaved with half 0 phase 2.  The
        # input stream leads by 2 segments so the first out-DMA's
        # semaphore wait (sync ring is in-order) never starves the
        # input DMAs.
        tps1 = tpsum.tile([128, MH], _F32, tag="tps")
        phase1_segment(1, 0, tps1)
        phase1_segment(1, 1, tps1)
        for s in range(N_SEG):
            phase2_mtile(0, s, ts0)
            if s + 2 < N_SEG:
                phase1_segment(1, s + 2, tps1)
        ts1 = reduce_cast(tps1)

        # Epilogue: phase 2 of half 1.
        for mt in range(N_MT_H):
            phase2_mtile(1, mt, ts1)


_NC_CACHE = None


def _get_nc():
    global _NC_CACHE
    if _NC_CACHE is not None:
        return _NC_CACHE
    nc = bacc.Bacc("TRN2", target_bir_lowering=False, debug=False)
    xt = nc.dram_tensor(
        "xt", [2 * N_SEG, 128, CC_SEG * MH], _BF16, kind="ExternalInput"
    ).ap()
    a_pre = nc.dram_tensor("a_pre", [128, N_KC * R], _BF16, kind="ExternalInput").ap()
    b_in = nc.dram_tensor("b_in", [LANE_P + R, N], _BF16, kind="ExternalInput").ap()
    out = nc.dram_tensor("out", [M_SHARD, N], _BF16, kind="ExternalOutput").ap()
    with tile.TileContext(nc) as tc:
        _build_kernel(tc, nc, xt, a_pre, b_in, out)
    nc.compile()
    _NC_CACHE = nc
    return nc


LAST_RESULTS = None


def kernel(x: np.ndarray, A: np.ndarray, B: np.ndarray) -> np.ndarray:
    global LAST_RESULTS
    assert x.shape == (B_DIM, SEQ, K), x.shape
    assert A.shape == (K, R), A.shape
    assert B.shape == (R, N), B.shape

    x_bf = np.asarray(x, dtype=np.float32).reshape(M_FULL, K).astype(ml_dtypes.bfloat16)
    a_np = np.asarray(A, dtype=np.float32)
    b_bf = (np.asarray(B, dtype=np.float32) * SCALING).astype(ml_dtypes.bfloat16)
    b_np = np.zeros((LANE_P + R, N), dtype=ml_dtypes.bfloat16)
    b_np[0:R] = b_bf
    b_np[LANE_P : LANE_P + R] = b_bf

    # Host pre-arrangement of A: [K, R] -> [128, (K/128) * R] bf16
    a_pre = np.ascontiguousarray(
        a_np.reshape(K // KC, KC, R).transpose(1, 0, 2).reshape(128, N_KC * R)
    ).astype(ml_dtypes.bfloat16)

    in_maps = []
    for i in range(NCORES):
        xT = x_bf[i * M_SHARD : (i + 1) * M_SHARD, :].T  # [K, M_SHARD]
        # [K, M] -> [s, cc, p, h, j] -> [h, s, p, cc, j] -> [16, 128, 4096]
        xt_i = np.ascontiguousarray(
            xT.reshape(N_SEG, CC_SEG, KC, 2, MH)
            .transpose(3, 0, 2, 1, 4)
            .reshape(2 * N_SEG, 128, CC_SEG * MH)
        )
        in_maps.append({"xt": xt_i, "a_pre": a_pre, "b_in": b_np})

    nc = _get_nc()
    trace = os.environ.get("KERNEL_TRACE", "0") == "1"
    tmpdir = os.environ.get("KERNEL_TMPDIR") or None
    res = run_bass_kernel_spmd(
        nc, in_maps, core_ids=list(range(NCORES)), trace=trace, tmpdir=tmpdir
    )
    LAST_RESULTS = res
    out = np.concatenate(
        [np.asarray(res.results[i]["out"], dtype=np.float32) for i in range(NCORES)],
        axis=0,
    )
    return out.reshape(B_DIM, SEQ, N)
